# revision 6
# baseline (speedup 1.0000x reference)
"""Single-head attention (nn_MultiHeadAttention) Trainium2 Bass kernel.

Full inputs: x [4, 2048, 1024], Wq/Wk/Wv/Wo [1024, 1024], biases [1024].
reference:  q = x @ Wq.T + bq ; k,v likewise
            scores = (q @ k.T) / sqrt(1024) ; attn = softmax(scores, -1)
            out = (attn @ v) @ Wo.T + bo

Sharding: 8 cores = 4 batches x 2 query-halves. Each core computes the
full K/V projection of its batch (duplicated across the pair) and
attention + output projection for its 1024 queries.

Host-side prep per core (b = c // 2, h = c % 2):
  xT = concat(x[b, h-half].T, x[b, other-half].T) -> [1024, 2048]
  (queries always occupy the first 1024 columns; the key order is a
   permutation, to which softmax attention is invariant)
  W*T = W*.T (so the contraction dim lands on SBUF partitions)

Per-core pipeline (matmuls in float32r = single-pass fp22 PE mode):
  V phase:  V[s,e]   = xT.T @ WvT (+bv)           -> resident SBUF
  K phase:  KT[e,s]  = WkT.T @ xT (+bk)           -> spilled to DRAM scratch
  Q phase:  QT[e,sq] = WqT.T @ xT[:, :1024] (+bq) -> resident
  scores:   uT[sk,sq] = exp((KT.T @ QT) / 32)     (no max-sub; |scores| < ~7)
            Z[1,sq] += ones.T @ uT                 (PE column-sum)
  Z:        PE-transpose 128-chunks of Z, reciprocal -> rZT[sq,1]
  ctx:      ctxT[e,sq] = V.T @ uT                  (V tiles stationary)
  out:      out[sq,f] = (ctxT.T @ WoT) * rZT + bo
"""

import numpy as np
from contextlib import ExitStack

import concourse.bass as bass
import concourse.bacc as bacc
import concourse.mybir as mybir
import concourse.tile as tile
from concourse import bass_utils
from concourse.masks import make_identity

F32 = mybir.dt.float32
F32R = mybir.dt.float32r
AF = mybir.ActivationFunctionType
ALU = mybir.AluOpType

B, S, D = 4, 2048, 1024
SQ = S // 2  # queries per core
N_CORES = 8


def build_nc(S=S, D=D, SQ=SQ):
    P = 128
    DT = D // P          # contraction tiles (8)
    ET = D // P          # output-dim tiles (8)
    NBW = min(512, D)    # free-dim block over D
    NB = D // NBW        # (2)
    SBW = min(512, S)    # free-dim block over S
    SKB = S // SBW       # (4)
    SKT = S // P         # key tiles (16)
    SQW = min(512, SQ)
    SQB = SQ // SQW      # (2)
    SQT = SQ // P        # query tiles (8)
    SCALE = 1.0 / float(np.sqrt(D))

    nc = bacc.Bacc("TRN2", target_bir_lowering=False, debug=False)

    xT = nc.dram_tensor("xT", [D, S], F32R, kind="ExternalInput")
    wqT = nc.dram_tensor("wqT", [D, D], F32R, kind="ExternalInput")
    wkT = nc.dram_tensor("wkT", [D, D], F32R, kind="ExternalInput")
    wvT = nc.dram_tensor("wvT", [D, D], F32R, kind="ExternalInput")
    woT = nc.dram_tensor("woT", [D, D], F32R, kind="ExternalInput")
    bqd = nc.dram_tensor("bq", [D], F32, kind="ExternalInput")
    bkd = nc.dram_tensor("bk", [D], F32, kind="ExternalInput")
    bvd = nc.dram_tensor("bv", [D], F32, kind="ExternalInput")
    bod = nc.dram_tensor("bo", [D], F32, kind="ExternalInput")
    outd = nc.dram_tensor("out", [SQ, D], F32, kind="ExternalOutput")

    def bcast_ap(handle):
        a = handle[:]
        return bass.AP(tensor=a.tensor, offset=a.offset, ap=[[0, P]] + list(a.ap))

    with tile.TileContext(nc) as tc, ExitStack() as top:
        singles = top.enter_context(tc.tile_pool(name="singles", bufs=1))
        dram = top.enter_context(tc.tile_pool(name="dram", bufs=1, space="DRAM"))
        psum_mm = top.enter_context(tc.tile_pool(name="psum_mm", bufs=4, space="PSUM"))
        psum_z = top.enter_context(tc.tile_pool(name="psum_z", bufs=2, space="PSUM"))
        psum_tr = top.enter_context(tc.tile_pool(name="psum_tr", bufs=2, space="PSUM"))

        ktd = dram.tile([D, S], F32R, name="ktd", tag="ktd")

        ones_f32 = singles.tile([P, 1], F32, name="ones_f32", tag="ones_f32")
        nc.vector.memset(ones_f32, 1.0)
        ones_col = singles.tile([P, 1], F32R, name="ones_col", tag="ones_col")
        nc.scalar.activation(out=ones_col, in_=ones_f32, func=AF.Copy)
        ident = singles.tile([P, P], F32, name="ident", tag="ident")
        make_identity(nc, ident)
        # per-partition bias layouts [p, t] = b[t*128 + p] (e on partitions)
        bq_pt = singles.tile([P, ET], F32, name="bq_pt", tag="bq_pt")
        nc.gpsimd.dma_start(out=bq_pt, in_=bqd[:].rearrange("(t p) -> p t", p=P))
        bk_pt = singles.tile([P, ET], F32, name="bk_pt", tag="bk_pt")
        nc.gpsimd.dma_start(out=bk_pt, in_=bkd[:].rearrange("(t p) -> p t", p=P))
        # broadcast bias layouts [128, D] (e on free dim)
        bv_bc = singles.tile([P, D], F32, name="bv_bc", tag="bv_bc")
        nc.gpsimd.dma_start(out=bv_bc, in_=bcast_ap(bvd))
        bo_bc = singles.tile([P, D], F32, name="bo_bc", tag="bo_bc")
        nc.gpsimd.dma_start(out=bo_bc, in_=bcast_ap(bod))
        rzt = singles.tile([P, SQT], F32, name="rzt", tag="rzt")

        v_pool = tc.alloc_tile_pool(name="v", bufs=SKT)
        v_tiles = [v_pool.tile([P, D], F32R, name=f"v{i}", tag="v") for i in range(SKT)]

        # ---------------- V / K / Q phases (xT resident) ----------------
        with tc.tile_pool(name="xt", bufs=DT) as xt_pool:
            xt_tiles = []
            xr = xT[:].rearrange("(t p) s -> t p s", p=P)
            for t in range(DT):
                xt_t = xt_pool.tile([P, S], F32R, name=f"xt{t}", tag="xt")
                nc.sync.dma_start(out=xt_t, in_=xr[t])
                xt_tiles.append(xt_t)

            # V phase
            with tc.tile_pool(name="wvcol", bufs=2) as wv_pool:
                for eb in range(NB):
                    wv_col = wv_pool.tile([P, DT, NBW], F32R, name="wv", tag="wv")
                    nc.sync.dma_start(
                        out=wv_col,
                        in_=wvT[:, eb * NBW:(eb + 1) * NBW].rearrange(
                            "(t p) e -> p t e", p=P),
                    )
                    for s in range(SKT):
                        pv = psum_mm.tile([P, NBW], F32, name="pv", tag="mm")
                        for d in range(DT):
                            nc.tensor.matmul(
                                pv,
                                lhsT=(xt_tiles[d][:, s * P:(s + 1) * P]),
                                rhs=(wv_col[:, d, :]),
                                start=(d == 0), stop=(d == DT - 1),
                            )
                        nc.vector.scalar_tensor_tensor(
                            out=v_tiles[s][:, eb * NBW:(eb + 1) * NBW],
                            in0=pv, scalar=1.0,
                            in1=bv_bc[:, eb * NBW:(eb + 1) * NBW],
                            op0=ALU.mult, op1=ALU.add,
                        )

            with tc.tile_pool(name="wcol", bufs=2) as wc_pool:
                # K phase -> DRAM scratch
                with tc.tile_pool(name="fly", bufs=3) as fly_pool:
                    for et in range(ET):
                        wk_col = wc_pool.tile([P, DT, P], F32R, name="wk", tag="wc")
                        nc.sync.dma_start(
                            out=wk_col,
                            in_=wkT[:, et * P:(et + 1) * P].rearrange(
                                "(t p) e -> p t e", p=P),
                        )
                        for sb in range(SKB):
                            pk = psum_mm.tile([P, SBW], F32, name="pk", tag="mm")
                            for d in range(DT):
                                nc.tensor.matmul(
                                    pk,
                                    lhsT=(wk_col[:, d, :]),
                                    rhs=(xt_tiles[d][:, sb * SBW:(sb + 1) * SBW]),
                                    start=(d == 0), stop=(d == DT - 1),
                                )
                            ktf = fly_pool.tile([P, SBW], F32R, name="ktf", tag="fly")
                            nc.scalar.activation(
                                out=ktf, in_=pk, func=AF.Identity,
                                bias=bk_pt[:, et:et + 1], scale=1.0,
                            )
                            nc.sync.dma_start(
                                out=ktd[et * P:(et + 1) * P, sb * SBW:(sb + 1) * SBW],
                                in_=ktf,
                            )

                # Q phase (queries = first SQ cols of xT)
                qt_pool = tc.alloc_tile_pool(name="qt", bufs=ET, side="right")
                qt_tiles = [qt_pool.tile([P, SQ], F32R, name=f"qt{i}", tag="qt")
                            for i in range(ET)]
                for et in range(ET):
                    wq_col = wc_pool.tile([P, DT, P], F32R, name="wq", tag="wc")
                    nc.sync.dma_start(
                        out=wq_col,
                        in_=wqT[:, et * P:(et + 1) * P].rearrange(
                            "(t p) e -> p t e", p=P),
                    )
                    for sb in range(SQB):
                        pq = psum_mm.tile([P, SQW], F32, name="pq", tag="mm")
                        for d in range(DT):
                            nc.tensor.matmul(
                                pq,
                                lhsT=(wq_col[:, d, :]),
                                rhs=(xt_tiles[d][:, sb * SQW:(sb + 1) * SQW]),
                                start=(d == 0), stop=(d == DT - 1),
                            )
                        nc.scalar.activation(
                            out=qt_tiles[et][:, sb * SQW:(sb + 1) * SQW],
                            in_=pq, func=AF.Identity,
                            bias=bq_pt[:, et:et + 1], scale=1.0,
                        )

        # ---------------- scores + Z (KT streamed back) ----------------
        u_pool = tc.alloc_tile_pool(name="u", bufs=SKT * SQB)
        u_tiles = [[None] * SKT for _ in range(SQB)]
        with tc.tile_pool(name="ktcol", bufs=2) as kt_pool:
            pz = [psum_z.tile([1, SQW], F32, name=f"pz{q}", tag="z")
                  for q in range(SQB)]
            for sk in range(SKT):
                kt_col = kt_pool.tile([P, ET, P], F32R, name="ktc", tag="ktc")
                nc.sync.dma_start(
                    out=kt_col,
                    in_=ktd[:, sk * P:(sk + 1) * P].rearrange("(t p) s -> p t s", p=P),
                )
                for q in range(SQB):
                    ps = psum_mm.tile([P, SQW], F32, name="ps", tag="mm")
                    for e in range(ET):
                        nc.tensor.matmul(
                            ps,
                            lhsT=(kt_col[:, e, :]),
                            rhs=(qt_tiles[e][:, q * SQW:(q + 1) * SQW]),
                            start=(e == 0), stop=(e == ET - 1),
                        )
                    ut = u_pool.tile([P, SQW], F32R, name=f"u{q}_{sk}", tag="u")
                    nc.scalar.activation(out=ut, in_=ps, func=AF.Exp, scale=SCALE)
                    u_tiles[q][sk] = ut
                    nc.tensor.matmul(
                        pz[q], lhsT=(ones_col), rhs=(ut),
                        start=(sk == 0), stop=(sk == SKT - 1),
                    )
            # Z -> 1/Z transposed to per-partition layout
            for q in range(SQB):
                z_sb = singles.tile([1, SQW], F32, name="z_sb", tag="z_sb")
                nc.scalar.copy(z_sb, pz[q])
                for j in range(SQW // P):
                    pt = psum_tr.tile([P, 1], F32, name="pt", tag="tr")
                    nc.tensor.transpose(
                        pt, z_sb[0:1, j * P:(j + 1) * P], ident[0:1, 0:1])
                    jj = q * (SQW // P) + j
                    nc.vector.reciprocal(out=rzt[:, jj:jj + 1], in_=pt)
        qt_pool.release()

        # ---------------- ctx phase ----------------
        ctx_pool = tc.alloc_tile_pool(name="ctx", bufs=ET, side="right")
        ctx_tiles = [ctx_pool.tile([P, SQ], F32R, name=f"ctx{i}", tag="ctx")
                     for i in range(ET)]
        for e in range(ET):
            for q in range(SQB):
                pc = psum_mm.tile([P, SQW], F32, name="pc", tag="mm")
                for sk in range(SKT):
                    nc.tensor.matmul(
                        pc,
                        lhsT=(v_tiles[sk][:, e * P:(e + 1) * P]),
                        rhs=(u_tiles[q][sk]),
                        start=(sk == 0), stop=(sk == SKT - 1),
                    )
                nc.scalar.copy(ctx_tiles[e][:, q * SQW:(q + 1) * SQW], pc)
        u_pool.release()
        v_pool.release()

        # ---------------- out projection ----------------
        with tc.tile_pool(name="wocol", bufs=2) as wo_pool, \
             tc.tile_pool(name="ofly", bufs=3) as o_pool:
            for fb in range(NB):
                wo_col = wo_pool.tile([P, DT, NBW], F32R, name="wo", tag="wo")
                nc.sync.dma_start(
                    out=wo_col,
                    in_=woT[:, fb * NBW:(fb + 1) * NBW].rearrange(
                        "(t p) f -> p t f", p=P),
                )
                for st in range(SQT):
                    po = psum_mm.tile([P, NBW], F32, name="po", tag="mm")
                    for e in range(ET):
                        nc.tensor.matmul(
                            po,
                            lhsT=(ctx_tiles[e][:, st * P:(st + 1) * P]),
                            rhs=(wo_col[:, e, :]),
                            start=(e == 0), stop=(e == ET - 1),
                        )
                    osb = o_pool.tile([P, NBW], F32, name="osb", tag="ofly")
                    nc.vector.scalar_tensor_tensor(
                        out=osb, in0=po, scalar=rzt[:, st:st + 1],
                        in1=bo_bc[:, fb * NBW:(fb + 1) * NBW],
                        op0=ALU.mult, op1=ALU.add,
                    )
                    nc.sync.dma_start(
                        out=outd[st * P:(st + 1) * P, fb * NBW:(fb + 1) * NBW],
                        in_=osb,
                    )
        ctx_pool.release()

    nc.compile()
    return nc


_NC_CACHE = {}


def _get_nc():
    if "nc" not in _NC_CACHE:
        _NC_CACHE["nc"] = build_nc()
    return _NC_CACHE["nc"]


def _round_f32r(a):
    """Round-to-nearest to fp32r precision (fp22 = s1e8m13), so the PE's
    read-truncation behaves like round-to-nearest overall."""
    u = np.ascontiguousarray(a, np.float32).view(np.uint32)
    u = ((u.astype(np.uint64) + 0x200) & 0xFFFFFC00).astype(np.uint32)
    return u.view(np.float32)


def make_in_maps(x, Wq, bq, Wk, bk, Wv, bv, Wo, bo):
    x = _round_f32r(np.asarray(x, dtype=np.float32))
    wqT = _round_f32r(np.asarray(Wq, np.float32).T)
    wkT = _round_f32r(np.asarray(Wk, np.float32).T)
    wvT = _round_f32r(np.asarray(Wv, np.float32).T)
    woT = _round_f32r(np.asarray(Wo, np.float32).T)
    bq = np.ascontiguousarray(np.asarray(bq, np.float32))
    bk = np.ascontiguousarray(np.asarray(bk, np.float32))
    bv = np.ascontiguousarray(np.asarray(bv, np.float32))
    bo = np.ascontiguousarray(np.asarray(bo, np.float32))

    in_maps = []
    for c in range(N_CORES):
        b, h = c // 2, c % 2
        xb = x[b]  # [S, D]
        mine = xb[h * SQ:(h + 1) * SQ]
        other = xb[(1 - h) * SQ:(2 - h) * SQ]
        xTc = np.ascontiguousarray(np.concatenate([mine, other], axis=0).T)
        in_maps.append({
            "xT": xTc, "wqT": wqT, "wkT": wkT, "wvT": wvT, "woT": woT,
            "bq": bq, "bk": bk, "bv": bv, "bo": bo,
        })
    return in_maps


def assemble(results):
    out = np.empty((B, S, D), np.float32)
    for c in range(N_CORES):
        b, h = c // 2, c % 2
        out[b, h * SQ:(h + 1) * SQ] = results[c]["out"]
    return out


def kernel(x, Wq, bq, Wk, bk, Wv, bv, Wo, bo, **kwargs):
    nc = _get_nc()
    in_maps = make_in_maps(x, Wq, bq, Wk, bk, Wv, bv, Wo, bo)
    res = bass_utils.run_bass_kernel_spmd(nc, in_maps, core_ids=list(range(N_CORES)))
    return assemble(res.results)
